# revision 1
# baseline (speedup 1.0000x reference)
"""Trainium2 Bass kernel for nn_DBLoss (YOLO-style detection loss).

Strategy (data parallel over batch, 8 cores, 2 images each):
  total = BOX_W * S_box/n_pos + OBJ_W*(S_sp_obj - S_obj_pos)/(B*na*H*W)
          + CLS_W * S_cls/(n_pos*NC)
  - S_sp_obj: dense softplus sum over the obj-logit channel of p_raw —
    the memory-bound part; each core streams its 13 MB shard through SBUF.
  - S_obj_pos/S_cls/S_box: only at "positive" cells. The positive-cell
    assignment (grid cell + anchor per label, 3x3 neighborhood, last-writer
    box, class-set union) depends only on the tiny label tensors and is
    computed on host; the per-cell predictions are gathered on device from
    the p_raw shard via indirect DMA and reduced there.
  - Each core returns [128, 25] partial sums; host sums and combines.
"""
import numpy as np

import concourse.bass as bass
import concourse.bacc as bacc
import concourse.tile as tile
from concourse import mybir
from concourse.bass_utils import run_bass_kernel_spmd

# problem constants (hardcoded per the task spec)
B, NA, H, W, D = 16, 3, 80, 80, 85
NC_CLS = 80
N = 48
STRIDE = 8.0
IMG_SIZE = 640.0
BOX_W, OBJ_W, CLS_W = 7.5, 1.0, 0.5
ANCHORS = np.array([[10.0, 13.0], [16.0, 30.0], [33.0, 23.0]], dtype=np.float32)

N_CORES = 8
B_SH = B // N_CORES              # images per core
CELLS = B_SH * NA * H * W        # 38400 p-rows per core
CPP = CELLS // 128               # 300 cells per partition
NJ = 7                           # gather groups: 128*7 = 896 slots >= 2*48*9
NSLOT = 128 * NJ
NCHUNK = 4                       # dense stream chunks
CHUNK_CELLS = [75, 75, 75, 75]
CHUNK_OFF = [0, 75, 150, 225]
assert sum(CHUNK_CELLS) == CPP

# meta field layout (each field is NJ columns wide)
F_VALID, F_CI8, F_CJ8, F_AW, F_AH, F_TX, F_TY, F_TX1, F_TX2, F_TY1, F_TY2, \
    F_AREAG, F_ATANT = range(13)
NFIELD = 13

f32 = np.float32
AF = mybir.ActivationFunctionType
ALU = mybir.AluOpType


# ---------------------------------------------------------------- host side

def _host_assign(labels_xywh, labels_cls):
    """Replicates the reference target assignment exactly (float32 numpy).

    Returns (per_image, n_pos) where per_image[b] is a tuple of arrays
    (cells, last_n, hot) with cells the sorted unique positive cell ids
    (a*H*W + j*W + i), last_n the last-writing label index per cell, and
    hot a [n_cells, NC_CLS] multi-hot of all classes written to the cell.
    """
    lab = labels_xywh.astype(np.float32) * f32(IMG_SIZE)          # [B,N,4]
    gx, gy, gw, gh = lab[..., 0], lab[..., 1], lab[..., 2], lab[..., 3]
    # NOTE: the neuron backend's f32->i32 convert rounds to nearest (RNE),
    # unlike numpy's astype truncation — match it, since the grading
    # reference runs on the same backend.
    gi = np.rint(np.clip(gx / f32(STRIDE), f32(0), f32(W - 0.001))).astype(np.int32)
    gj = np.rint(np.clip(gy / f32(STRIDE), f32(0), f32(H - 0.001))).astype(np.int32)
    a_wh = ANCHORS / f32(STRIDE)
    gtw = (gw / f32(STRIDE)).astype(np.float32)
    gth = (gh / f32(STRIDE)).astype(np.float32)
    inter = np.minimum(gtw[..., None], a_wh[:, 0]) * np.minimum(gth[..., None], a_wh[:, 1])
    union = gtw[..., None] * gth[..., None] + a_wh[:, 0] * a_wh[:, 1] - inter + f32(1e-9)
    best_a = np.argmax((inter / union).astype(np.float32), axis=-1).astype(np.int32)

    # offsets in the reference's order: di over x (outer), dj over y (inner)
    di = np.array([-1, -1, -1, 0, 0, 0, 1, 1, 1], dtype=np.int32)
    dj = np.array([-1, 0, 1, -1, 0, 1, -1, 0, 1], dtype=np.int32)
    nof = np.repeat(np.arange(N, dtype=np.int64), 9)

    per_image = []
    n_pos = 0
    lc = np.asarray(labels_cls).astype(np.int64)
    for b in range(B):
        ii = np.clip(gi[b][:, None] + di[None, :], 0, W - 1)
        jj = np.clip(gj[b][:, None] + dj[None, :], 0, H - 1)
        cell = (best_a[b][:, None].astype(np.int64) * H + jj) * W + ii     # [N,9]
        cellf = cell.ravel()
        u_cells, inv = np.unique(cellf, return_inverse=True)
        last_n = np.zeros(len(u_cells), dtype=np.int64)
        np.maximum.at(last_n, inv, nof)
        pair = cellf * NC_CLS + lc[b][nof]
        u_pairs = np.unique(pair)
        hot = np.zeros((len(u_cells), NC_CLS), dtype=np.float32)
        slot_of_pair = np.searchsorted(u_cells, u_pairs // NC_CLS)
        hot[slot_of_pair, u_pairs % NC_CLS] = 1.0
        per_image.append((u_cells, last_n, hot))
        n_pos += len(u_cells)
    return lab, per_image, n_pos


def _host_build_core_inputs(lab, per_image, core):
    """Builds idx [128,NJ] i32, meta [128,NFIELD*NJ] f32, hot [128,NJ*NC] f32
    for one core. Device slot s=(p,jcol) holds host slot jcol*128+p."""
    idx_s = np.zeros(NSLOT, dtype=np.int32)
    meta_s = np.zeros((NSLOT, NFIELD), dtype=np.float32)
    hot_s = np.zeros((NSLOT, NC_CLS), dtype=np.float32)
    # safe defaults for padding slots (avoid div-by-tiny; valid=0 masks them)
    meta_s[:, F_AW] = 10.0
    meta_s[:, F_AH] = 13.0
    meta_s[:, F_TX2] = 1.0
    meta_s[:, F_TY2] = 1.0
    meta_s[:, F_AREAG] = 1.0

    s = 0
    for li in range(B_SH):
        b = core * B_SH + li
        u_cells, last_n, hot = per_image[b]
        n = len(u_cells)
        assert s + n <= NSLOT
        sl = slice(s, s + n)
        a = u_cells // (H * W)
        j = (u_cells % (H * W)) // W
        i = u_cells % W
        idx_s[sl] = (li * NA * H * W + u_cells).astype(np.int32)
        meta_s[sl, F_VALID] = 1.0
        meta_s[sl, F_CI8] = (i * f32(STRIDE)).astype(np.float32)
        meta_s[sl, F_CJ8] = (j * f32(STRIDE)).astype(np.float32)
        meta_s[sl, F_AW] = ANCHORS[a, 0]
        meta_s[sl, F_AH] = ANCHORS[a, 1]
        tb = lab[b, last_n].astype(np.float32)                   # [n,4]
        tx, ty, tw, th = tb[:, 0], tb[:, 1], tb[:, 2], tb[:, 3]
        half = f32(0.5)
        tx1, tx2 = tx - tw * half, tx + tw * half
        ty1, ty2 = ty - th * half, ty + th * half
        meta_s[sl, F_TX] = tx
        meta_s[sl, F_TY] = ty
        meta_s[sl, F_TX1] = tx1
        meta_s[sl, F_TX2] = tx2
        meta_s[sl, F_TY1] = ty1
        meta_s[sl, F_TY2] = ty2
        meta_s[sl, F_AREAG] = np.maximum(tx2 - tx1, 0) * np.maximum(ty2 - ty1, 0)
        meta_s[sl, F_ATANT] = np.arctan(tw / (th + f32(1e-7)))
        hot_s[sl] = hot
        s += n

    # host slot s -> device (partition p = s%128, column jcol = s//128)
    idx_dev = idx_s.reshape(NJ, 128).T.copy()                    # [128, NJ]
    # meta: field-major columns: dev[:, f*NJ + jcol]
    m = meta_s.reshape(NJ, 128, NFIELD).transpose(1, 2, 0)       # [128,NFIELD,NJ]
    meta_dev = np.ascontiguousarray(m.reshape(128, NFIELD * NJ))
    h = hot_s.reshape(NJ, 128, NC_CLS).transpose(1, 0, 2)        # [128,NJ,NC]
    hot_dev = np.ascontiguousarray(h.reshape(128, NJ * NC_CLS))
    return idx_dev, meta_dev, hot_dev


# ------------------------------------------------------------- device build

ATAN_C = [9.999966198e-01, -3.330530727e-01, 1.961716862e-01,
          -1.229207765e-01, 5.959836087e-02, -1.440560854e-02]
# NOTE: a single indirect_dma_start with a [128,NJ] offset AP produces a
# DIFFERENT offset<->destination pairing on hardware than in CoreSim
# (rows end up scrambled across slots). One call per column with a [128,1]
# offset AP (the production tile_scatter_add pattern) is correct on HW.
GATHER_SINGLE = False


def _build_device_kernel(tc, p, idx_d, meta_d, hot_d, out_d):
    nc = tc.nc
    dt = mybir.dt.float32
    import contextlib
    with contextlib.ExitStack() as ctx:
        sp = ctx.enter_context(tc.tile_pool(name="stream", bufs=NCHUNK))
        sm = ctx.enter_context(tc.tile_pool(name="small", bufs=1))

        # ---- small inputs. idx gates the gather chain: sync ring ahead of
        # the stream (FIFO per ring -> drains first, ~1.3us). meta/hot on
        # the ACT ring; they may straggle mid-stream but are only needed
        # ~20us in by the DVE chain.
        idx_t = sm.tile([128, NJ], mybir.dt.int32, name="idx_t")
        nc.sync.dma_start(idx_t[:], idx_d.ap())
        meta_t = sm.tile([128, NFIELD * NJ], dt, name="meta_t")
        nc.scalar.dma_start(meta_t[:], meta_d.ap())
        hot_t = sm.tile([128, NJ * NC_CLS], dt, name="hot_t")
        nc.scalar.dma_start(hot_t[:], hot_d.ap())

        def F(f):
            return meta_t[:, f * NJ:(f + 1) * NJ]

        # ---- dense stream (sync/SP HWDGE ring, bufs=NCHUNK so all chunk
        # DMAs are in flight immediately and never stall on compute)
        p_ap = p.ap()
        p_t = p_ap.rearrange("(pp c) d -> pp (c d)", pp=128)      # [128, CPP*D]
        # whole stream on the sync ring: a second HWDGE ring drains
        # unpredictably slower against it (measured), so don't split
        chunks = []
        for k in range(NCHUNK):
            ch_t = sp.tile([128, CHUNK_CELLS[k] * D], dt, tag="chunk",
                           name=f"chunk{k}")
            nc.sync.dma_start(
                ch_t[:],
                p_t[:, CHUNK_OFF[k] * D:(CHUNK_OFF[k] + CHUNK_CELLS[k]) * D])
            chunks.append(ch_t)

        # ---- gather positive-cell rows from HBM (SWDGE indirect)
        rows = sm.tile([128, NJ * D], dt, name="rows")
        if GATHER_SINGLE:
            nc.gpsimd.indirect_dma_start(
                out=rows[:],
                out_offset=None,
                in_=p_ap,
                in_offset=bass.IndirectOffsetOnAxis(ap=idx_t[:], axis=0),
            )
        else:
            for jcol in range(NJ):
                nc.gpsimd.indirect_dma_start(
                    out=rows[:, jcol * D:(jcol + 1) * D],
                    out_offset=None,
                    in_=p_ap,
                    in_offset=bass.IndirectOffsetOnAxis(
                        ap=idx_t[:, jcol:jcol + 1], axis=0),
                )
        rows_r = rows[:].rearrange("p (j c) -> p j c", c=D)       # [128,NJ,D]

        def CH(c):
            return rows_r[:, :, c]                                 # [128,NJ]

        T = lambda name: sm.tile([128, NJ], dt, name=name)

        # ---- sparse ACT: everything on the natural_log_exp table set.
        # sigmoid(x) = 1/(1+exp(-x)); softplus(x) = ln(exp(x) + 1) with the
        # +1 folded into Ln's bias; arctan via DVE polynomial. Only two
        # table loads in the whole kernel (exp at start, ln at the end).
        e0, e1, ew, eh = T("e0"), T("e1"), T("ew"), T("eh")
        nc.scalar.activation(e0[:], CH(0), AF.Exp, scale=-1.0)
        nc.scalar.activation(e1[:], CH(1), AF.Exp, scale=-1.0)
        nc.scalar.activation(ew[:], CH(2), AF.Exp)
        nc.scalar.activation(eh[:], CH(3), AF.Exp)
        # exp of cls logits into the shared exp buffer (cols CPP:CPP+560)
        expbuf = sm.tile([128, CPP + NJ * NC_CLS], dt, name="expbuf")
        cls_in = rows_r[:, :, 5:5 + NC_CLS]                       # [128,NJ,NC]
        ecls_r = expbuf[:, CPP:].rearrange("p (j c) -> p j c", c=NC_CLS)
        nc.scalar.activation(ecls_r, cls_in, AF.Exp)

        v = nc.vector
        sx, sy = T("sx"), T("sy")
        v.tensor_scalar_add(e0[:], e0[:], 1.0)
        v.reciprocal(sx[:], e0[:])
        v.tensor_scalar_add(e1[:], e1[:], 1.0)
        v.reciprocal(sy[:], e1[:])

        pw, ph = T("pw"), T("ph")
        v.tensor_mul(pw[:], ew[:], F(F_AW))
        v.tensor_mul(ph[:], eh[:], F(F_AH))

        px, py = T("px"), T("py")
        v.tensor_scalar_mul(sx[:], sx[:], STRIDE)
        v.tensor_add(px[:], sx[:], F(F_CI8))
        v.tensor_scalar_mul(sy[:], sy[:], STRIDE)
        v.tensor_add(py[:], sy[:], F(F_CJ8))

        pwh, phh = T("pwh"), T("phh")
        v.tensor_scalar_mul(pwh[:], pw[:], 0.5)
        v.tensor_scalar_mul(phh[:], ph[:], 0.5)
        px1, px2, py1, py2 = T("px1"), T("px2"), T("py1"), T("py2")
        v.tensor_sub(px1[:], px[:], pwh[:])
        v.tensor_add(px2[:], px[:], pwh[:])
        v.tensor_sub(py1[:], py[:], phh[:])
        v.tensor_add(py2[:], py[:], phh[:])

        # arctan(pw / (ph + eps)) via z = min(r, 1/r) and a degree-5
        # minimax polynomial in z^2 (max abs err ~1.1e-5 on [0,1]):
        # at = az + m*(pi/2 - 2*az), m = (r > 1).
        t0, t1 = T("t0"), T("t1")
        v.tensor_scalar_add(t0[:], ph[:], 1e-7)
        v.reciprocal(t0[:], t0[:])
        v.tensor_mul(t0[:], t0[:], pw[:])                         # r
        rinv, zmin, m = T("rinv"), T("zmin"), T("m")
        v.reciprocal(rinv[:], t0[:])
        v.tensor_tensor(zmin[:], t0[:], rinv[:], op=ALU.min)
        v.tensor_scalar(m[:], t0[:], 1.0, None, op0=ALU.is_gt)
        u, at = T("u"), T("at")
        v.tensor_mul(u[:], zmin[:], zmin[:])
        v.tensor_scalar(at[:], u[:], ATAN_C[5], ATAN_C[4],
                        op0=ALU.mult, op1=ALU.add)
        for c in (ATAN_C[3], ATAN_C[2], ATAN_C[1], ATAN_C[0]):
            v.tensor_mul(at[:], at[:], u[:])
            v.tensor_scalar_add(at[:], at[:], c)
        v.tensor_mul(at[:], at[:], zmin[:])                       # atan(z)
        v.tensor_mul(t1[:], at[:], m[:])
        v.tensor_scalar_mul(t1[:], t1[:], 2.0)
        v.tensor_sub(at[:], at[:], t1[:])
        v.tensor_scalar_mul(m[:], m[:], float(np.pi / 2))
        v.tensor_add(at[:], at[:], m[:])

        # intersection / union / iou
        iw, ih = T("iw"), T("ih")
        v.tensor_tensor(t0[:], px2[:], F(F_TX2), op=ALU.min)
        v.tensor_tensor(t1[:], px1[:], F(F_TX1), op=ALU.max)
        v.tensor_sub(iw[:], t0[:], t1[:])
        v.tensor_scalar_max(iw[:], iw[:], 0.0)
        v.tensor_tensor(t0[:], py2[:], F(F_TY2), op=ALU.min)
        v.tensor_tensor(t1[:], py1[:], F(F_TY1), op=ALU.max)
        v.tensor_sub(ih[:], t0[:], t1[:])
        v.tensor_scalar_max(ih[:], ih[:], 0.0)
        inter = T("inter")
        v.tensor_mul(inter[:], iw[:], ih[:])
        un = T("un")
        v.tensor_mul(un[:], pw[:], ph[:])
        v.tensor_add(un[:], un[:], F(F_AREAG))
        v.tensor_sub(un[:], un[:], inter[:])
        v.tensor_scalar_add(un[:], un[:], 1e-7)
        iou = T("iou")
        v.reciprocal(un[:], un[:])
        v.tensor_mul(iou[:], inter[:], un[:])

        # enclosing box diag, center distance
        cw, chh = T("cw"), T("chh")
        v.tensor_tensor(t0[:], px2[:], F(F_TX2), op=ALU.max)
        v.tensor_tensor(t1[:], px1[:], F(F_TX1), op=ALU.min)
        v.tensor_sub(cw[:], t0[:], t1[:])
        v.tensor_tensor(t0[:], py2[:], F(F_TY2), op=ALU.max)
        v.tensor_tensor(t1[:], py1[:], F(F_TY1), op=ALU.min)
        v.tensor_sub(chh[:], t0[:], t1[:])
        cc = T("cc")
        v.tensor_mul(cw[:], cw[:], cw[:])
        v.tensor_mul(chh[:], chh[:], chh[:])
        v.tensor_add(cc[:], cw[:], chh[:])
        v.tensor_scalar_add(cc[:], cc[:], 1e-7)
        v.reciprocal(cc[:], cc[:])
        rho2 = T("rho2")
        v.tensor_sub(t0[:], px[:], F(F_TX))
        v.tensor_sub(t1[:], py[:], F(F_TY))
        v.tensor_mul(t0[:], t0[:], t0[:])
        v.tensor_mul(t1[:], t1[:], t1[:])
        v.tensor_add(rho2[:], t0[:], t1[:])
        v.tensor_mul(rho2[:], rho2[:], cc[:])                    # rho2/c2

        # v-term and alpha
        vv = T("vv")
        v.tensor_sub(vv[:], F(F_ATANT), at[:])
        v.tensor_mul(vv[:], vv[:], vv[:])
        v.tensor_scalar_mul(vv[:], vv[:], float(4.0 / np.pi**2))
        ad = T("ad")
        v.tensor_sub(ad[:], vv[:], iou[:])
        v.tensor_scalar_add(ad[:], ad[:], 1.0 + 1e-7)
        v.reciprocal(ad[:], ad[:])
        v.tensor_mul(ad[:], ad[:], vv[:])                        # alpha
        v.tensor_mul(ad[:], ad[:], vv[:])                        # alpha*v

        term = T("term")
        v.tensor_sub(term[:], rho2[:], iou[:])                    # rho2/c2 - iou
        v.tensor_add(term[:], term[:], ad[:])
        v.tensor_scalar_add(term[:], term[:], 1.0)

        # ---- outputs tile: [0]=dense softplus sum, [1:8]=obj, [8:15]=cls,
        # [15:22]=box
        outv = sm.tile([128, 22], dt, name="outv")
        v.tensor_mul(outv[:, 15:22], term[:], F(F_VALID))
        v.tensor_mul(outv[:, 1:8], CH(4), F(F_VALID))

        # hot*x reduced per slot — independent of Ln, do it early
        hx = sm.tile([128, NJ * NC_CLS], dt, name="hx")
        hx_r = hx[:].rearrange("p (j c) -> p j c", c=NC_CLS)
        v.tensor_mul(hx_r, hot_t[:].rearrange("p (j c) -> p j c", c=NC_CLS),
                     cls_in)
        hxr = T("hxr")
        v.reduce_sum(hxr[:], hx_r, axis=mybir.AxisListType.X)

        # ---- dense obj: exp of channel 4 of each chunk into expbuf[0:CPP]
        for k in range(NCHUNK):
            ch_r = chunks[k][:].rearrange("p (c d) -> p c d", d=D)
            nc.scalar.activation(
                expbuf[:, CHUNK_OFF[k]:CHUNK_OFF[k] + CHUNK_CELLS[k]],
                ch_r[:, :, 4], AF.Exp)

        # ---- Ln passes with bias=1: ln(exp(x)+1)
        scr_d = sm.tile([128, CPP], dt, name="scr_d")
        nc.scalar.activation(scr_d[:], expbuf[:, :CPP], AF.Ln, bias=1.0,
                             accum_out=outv[:, 0:1])
        bce = sm.tile([128, NJ * NC_CLS], dt, name="bce")
        nc.scalar.activation(bce[:], expbuf[:, CPP:], AF.Ln, bias=1.0)
        clsr = T("clsr")
        v.reduce_sum(clsr[:], bce[:].rearrange("p (j c) -> p j c", c=NC_CLS),
                     axis=mybir.AxisListType.X)
        v.tensor_sub(clsr[:], clsr[:], hxr[:])
        v.tensor_mul(outv[:, 8:15], clsr[:], F(F_VALID))

        nc.scalar.dma_start(out_d.ap(), outv[:])


_NC_CACHE = {}


def _patch_act_tables():
    """Force Exp and Ln onto the combined natural_log_exp set so the kernel
    needs exactly one ACT table load (no mid-kernel or tail reloads). The
    pass picks the first set containing the function; IDs are positional,
    so strip Exp/Ln from every other set rather than reordering."""
    if getattr(bacc, "_dbloss_act_patch", False):
        return
    orig = bacc.get_activation_tables
    EXP, LN = AF.Exp, AF.Ln

    def patched(arch):
        tabs = dict(orig(arch))
        comb = next((name for name, fns in tabs.items()
                     if EXP in fns and LN in fns), None)
        if comb is not None:
            for name in tabs:
                if name != comb:
                    tabs[name] = {f for f in tabs[name] if f not in (EXP, LN)}
        return tabs

    bacc.get_activation_tables = patched
    bacc._dbloss_act_patch = True


def _get_compiled():
    if "nc" in _NC_CACHE:
        return _NC_CACHE["nc"]
    _patch_act_tables()
    nc = bacc.Bacc("TRN2", target_bir_lowering=False, debug=False,
                   num_devices=N_CORES)
    p = nc.dram_tensor("p", [CELLS, D], mybir.dt.float32, kind="ExternalInput")
    idx_d = nc.dram_tensor("idx", [128, NJ], mybir.dt.int32, kind="ExternalInput")
    meta_d = nc.dram_tensor("meta", [128, NFIELD * NJ], mybir.dt.float32,
                            kind="ExternalInput")
    hot_d = nc.dram_tensor("hot", [128, NJ * NC_CLS], mybir.dt.float32,
                           kind="ExternalInput")
    out_d = nc.dram_tensor("out", [128, 22], mybir.dt.float32,
                           kind="ExternalOutput")
    with tile.TileContext(nc) as tc:
        _build_device_kernel(tc, p, idx_d, meta_d, hot_d, out_d)
    nc.compile()
    _NC_CACHE["nc"] = nc
    return nc


def _make_in_maps(p_raw, labels_xywh, labels_cls):
    lab, per_image, n_pos = _host_assign(labels_xywh, labels_cls)
    p_flat = np.ascontiguousarray(p_raw, dtype=np.float32).reshape(B, NA * H * W, D)
    in_maps = []
    for core in range(N_CORES):
        idx_dev, meta_dev, hot_dev = _host_build_core_inputs(lab, per_image, core)
        p_shard = p_flat[core * B_SH:(core + 1) * B_SH].reshape(CELLS, D)
        in_maps.append({"p": p_shard, "idx": idx_dev, "meta": meta_dev,
                        "hot": hot_dev})
    return in_maps, n_pos


def _combine(results, n_pos):
    S_sp = S_obj = S_cls = S_box = 0.0
    for r in results:
        o = np.asarray(r["out"], dtype=np.float64)
        S_sp += o[:, 0:1].sum()
        S_obj += o[:, 1:8].sum()
        S_cls += o[:, 8:15].sum()
        S_box += o[:, 15:22].sum()
    npos = float(max(n_pos, 1))
    l_box = S_box / npos
    l_obj = (S_sp - S_obj) / float(B * NA * H * W)
    l_cls = S_cls / (npos * NC_CLS)
    return np.float32(BOX_W * l_box + OBJ_W * l_obj + CLS_W * l_cls)


def kernel(p_raw, labels_xywh, labels_cls):
    p_raw = np.asarray(p_raw, dtype=np.float32)
    labels_xywh = np.asarray(labels_xywh, dtype=np.float32)
    labels_cls = np.asarray(labels_cls)
    in_maps, n_pos = _make_in_maps(p_raw, labels_xywh, labels_cls)
    nc = _get_compiled()
    res = run_bass_kernel_spmd(nc, in_maps, core_ids=list(range(N_CORES)))
    return _combine(res.results, n_pos)


if __name__ == "__main__":
    import reference as R
    inputs = R.setup_inputs()
    inputs = {k: np.asarray(v) for k, v in inputs.items()}
    got = kernel(**inputs)
    print("kernel:", got)

